# revision 1
# baseline (speedup 1.0000x reference)
"""Trainium2 Bass kernel for nn_Attention_KV (dense transformer attention
with K=Q sharing and a linear positional bias), distributed over 8 cores.

Sharding: 2 batch-groups x 4 query-quarters (collective-free). Core
c = 4*g + s owns batches 4g..4g+3 and query rows i in [256*s, 256*(s+1)).
The positional bias pos_bias(i,j) is head/batch independent but sharded
by i-quarter, so each core loads exactly the pos slice it consumes —
no AllGather (measured ~300us fixed latency per collective on this
fabric, more than the whole rest of the kernel). The price is computing
k/v for 4 batches per core (cheap PE work) instead of 1.

All attention math keeps scores TRANSPOSED (keys j on partitions,
queries i on the free axis). Because dots = k @ k^T is symmetric this
costs nothing, and it makes softmax + the attn @ v contraction
expressible without any on-chip transpose:
  - scores^T lands directly in PSUM: a K=64 dots matmul plus an identity
    matmul that adds pos_bias^T/c (pos is pre-divided by
    c = scale*sum(w_pos) on-device, so exp(scale=c) on the Scalar engine
    applies both the dot-product scaling and the bias in one pass)
  - attn@v as lhsT = v_ext (with a ones column appended -> row 64 of the
    result is the softmax denominator Z), rhs = exp(scores^T)
  - normalization folded into the PSUM->SBUF copy of U
pos flows in bf16 (softmax averaging damps its rounding error to ~1e-4
relative on the output); matmuls run in float32r. b_pos (a scalar added
to every score) is dropped: softmax is shift invariant.
"""

import sys

sys.path.insert(0, "/opt/trn_rl_repo")

import numpy as np

import concourse.bacc as bacc
import concourse.bass as bass
import concourse.mybir as mybir
from concourse import tile
from concourse.bass_utils import run_bass_kernel_spmd

B, N, DIM, H, POS_DIM = 8, 1024, 512, 8, 50
D = DIM // H  # 64
NC = 8  # cores
BPC = 4  # batches per core
IQ = 256  # query rows per core
JT = N // 128  # 8 j-tiles
SCALE = float(DIM) ** -0.5

F32 = mybir.dt.float32
F32R = mybir.dt.float32r
BF16 = mybir.dt.bfloat16
AX = mybir.AxisListType
ALU = mybir.AluOpType
ACTF = mybir.ActivationFunctionType

POS_CHUNK = 64  # i-columns of pos processed per DVE reduce


def build_program(reps: int = 1):
    nc = bacc.Bacc("TRN2", target_bir_lowering=False, debug=False)

    # ---- DRAM parameters (per-core) ----
    xT_d = nc.declare_dram_parameter("xT", [BPC, DIM, N], F32R, isOutput=False)
    xqT_d = nc.declare_dram_parameter("xqT", [BPC, DIM, IQ], F32R, isOutput=False)
    wkvT_d = nc.declare_dram_parameter("wkvT", [DIM, 2 * DIM], F32R, isOutput=False)
    wout_d = nc.declare_dram_parameter("wout", [DIM, DIM], F32R, isOutput=False)
    bout_d = nc.declare_dram_parameter("bout", [1, DIM], F32R, isOutput=False)
    wposr_d = nc.declare_dram_parameter(
        "wposr", [128, POS_CHUNK, POS_DIM], BF16, isOutput=False
    )
    posT_d = nc.declare_dram_parameter("posT", [N, IQ, POS_DIM], BF16, isOutput=False)
    ones_d = nc.declare_dram_parameter("ones", [128, 128], F32R, isOutput=False)
    id_d = nc.declare_dram_parameter("idm", [128, 128], BF16, isOutput=False)
    y_d = nc.declare_dram_parameter("y", [BPC, IQ, DIM], F32, isOutput=True)

    with tile.TileContext(nc) as tc:
        with (
            tc.tile_pool(name="persist", bufs=1) as pp,
            tc.tile_pool(name="pos_in", bufs=2) as pos_pool,
            tc.tile_pool(name="exps", bufs=3) as epool,
            tc.tile_pool(name="outsb", bufs=2) as opool,
            tc.tile_pool(name="mm_ps", bufs=2, space="PSUM") as mmps,
            tc.tile_pool(name="dots_ps", bufs=2, space="PSUM") as dotsps,
            tc.tile_pool(name="up_ps", bufs=2, space="PSUM") as upps,
            tc.tile_pool(name="dram", bufs=1, space="DRAM") as dram,
        ):
            for _rep in range(reps):
                # ---- preload small tensors + weights ----
                wposr = pp.tile([128, POS_CHUNK, POS_DIM], BF16, tag="wposr")
                nc.sync.dma_start(wposr[:], wposr_d[:])
                ones1 = pp.tile([1, 128], F32R, tag="ones1")
                nc.sync.dma_start(ones1[:], ones_d[0:1, :])
                idm = pp.tile([128, 128], BF16, tag="idm")
                nc.sync.dma_start(idm[:], id_d[:])
                wkvT = [
                    pp.tile([128, 2 * DIM], F32R, name=f"wkvT{t}", tag=f"wkvT{t}")
                    for t in range(4)
                ]
                for t in range(4):
                    nc.sync.dma_start(wkvT[t][:], wkvT_d[t * 128 : (t + 1) * 128, :])
                wout = [
                    pp.tile([64, DIM], F32R, name=f"wout{h}", tag=f"wout{h}")
                    for h in range(H)
                ]
                for h in range(H):
                    nc.sync.dma_start(wout[h][:], wout_d[h * 64 : (h + 1) * 64, :])
                bout = pp.tile([1, DIM], F32R, tag="bout")
                nc.sync.dma_start(bout[:], bout_d[:])

                # c = scale * sum(w_pos) on every partition; wposr /= c so the
                # pos-bias accumulates pre-divided and exp(scale=c) restores it.
                c_ap = pp.tile([128, 1], F32, tag="c_ap")
                ic_ap = pp.tile([128, 1], F32, tag="ic_ap")
                nc.vector.tensor_reduce(c_ap[:], wposr[:, 0, :], axis=AX.X, op=ALU.add)
                nc.scalar.mul(c_ap[:], c_ap[:], SCALE)
                nc.vector.reciprocal(ic_ap[:], c_ap[:])
                with nc.allow_low_precision(reason="w_pos/c in bf16 is intended"):
                    nc.vector.tensor_scalar_mul(wposr[:], wposr[:], ic_ap[:])

                # posT_sb[jt] = pos_bias^T / c for key-tile jt (128 j x 256 i)
                posT_sb = [
                    pp.tile([128, IQ], BF16, name=f"posT{j}", tag=f"posT{j}")
                    for j in range(JT)
                ]
                def emit_pos():
                    # ---- pos-bias phase: all j, this core's i-quarter ----

                    for jt in range(JT):
                        for ic in range(IQ // POS_CHUNK):
                            sl = slice(ic * POS_CHUNK, (ic + 1) * POS_CHUNK)
                            pt = pos_pool.tile(
                                [128, POS_CHUNK, POS_DIM], BF16, name="pchunk", tag="pchunk"
                            )
                            nc.sync.dma_start(
                                pt[:], posT_d[jt * 128 : (jt + 1) * 128, sl, :]
                            )
                            nc.vector.tensor_tensor(pt[:], pt[:], wposr[:], op=ALU.mult)
                            with nc.allow_low_precision(
                                reason="pos bias flows in bf16 by design"
                            ):
                                nc.vector.tensor_reduce(
                                    posT_sb[jt][:, sl], pt[:], axis=AX.X, op=ALU.add
                                )


                rzb = pp.tile([64, IQ], F32, tag="rzb")
                rzrow = pp.tile([65, IQ], F32, tag="rzrow")
                rz_bounce = dram.tile([1, IQ], F32)

                # ---- per batch: kv, attention, projection ----
                UT_sets = {
                    s2: [
                        pp.tile([64, IQ], F32R, name=f"UT{h}_{s2}", tag=f"UT{h}_{s2}")
                        for h in range(H)
                    ]
                    for s2 in (0, 1)
                }
                kv_tiles = {}

                def emit_kv(b):
                    s2 = b % 2  # double-buffer set for cross-batch overlap
                    xT = [
                        pp.tile([128, N], F32R, name=f"xT{t}_{s2}", tag=f"xT{t}_{s2}")
                        for t in range(4)
                    ]
                    for t in range(4):
                        nc.sync.dma_start(
                            xT[t][:], xT_d[b, t * 128 : (t + 1) * 128, :]
                        )
                    xqT = [
                        pp.tile(
                            [128, IQ], F32R, name=f"xqT{t}_{s2}", tag=f"xqT{t}_{s2}"
                        )
                        for t in range(4)
                    ]
                    for t in range(4):
                        nc.sync.dma_start(
                            xqT[t][:], xqT_d[b, t * 128 : (t + 1) * 128, :]
                        )

                    kT = [
                        pp.tile([128, N], F32R, name=f"kT{t}_{s2}", tag=f"kT{t}_{s2}")
                        for t in range(4)
                    ]
                    for t in range(4):
                        for nchunk in range(2):
                            ps = mmps.tile([128, 512], F32, name="mmtile", tag="mm")
                            for dc in range(4):
                                nc.tensor.matmul(
                                    ps[:],
                                    wkvT[dc][:, t * 128 : (t + 1) * 128],
                                    xT[dc][:, nchunk * 512 : (nchunk + 1) * 512],
                                    start=(dc == 0),
                                    stop=(dc == 3),
                                )
                            nc.vector.tensor_copy(
                                kT[t][:, nchunk * 512 : (nchunk + 1) * 512], ps[:]
                            )
                    kQT = [
                        pp.tile(
                            [128, IQ], F32R, name=f"kQT{t}_{s2}", tag=f"kQT{t}_{s2}"
                        )
                        for t in range(4)
                    ]
                    for t in range(4):
                        ps = mmps.tile([128, IQ], F32, name="mmq", tag="mm")
                        for dc in range(4):
                            nc.tensor.matmul(
                                ps[:],
                                wkvT[dc][:, t * 128 : (t + 1) * 128],
                                xqT[dc][:],
                                start=(dc == 0),
                                stop=(dc == 3),
                            )
                        nc.vector.tensor_copy(kQT[t][:], ps[:])

                    vext = [
                        pp.tile(
                            [128, H, D + 1],
                            F32R,
                            name=f"vext{t}_{s2}",
                            tag=f"vext{t}_{s2}",
                        )
                        for t in range(JT)
                    ]
                    for nt in range(JT):
                        ps = mmps.tile([128, 512], F32, name="mmtile", tag="mm")
                        for dc in range(4):
                            nc.tensor.matmul(
                                ps[:],
                                xT[dc][:, nt * 128 : (nt + 1) * 128],
                                wkvT[dc][:, DIM : 2 * DIM],
                                start=(dc == 0),
                                stop=(dc == 3),
                            )
                        nc.sync.dma_start(vext[nt][:, :, D : D + 1], ones_d[:, 0:H])
                        nc.vector.tensor_copy(
                            vext[nt][:, :, 0:D],
                            ps[:].rearrange("p (h d) -> p h d", h=H),
                        )
                    kv_tiles[b] = (kT, kQT, vext)

                def emit_attn(b):
                    s2 = b % 2
                    kT, kQT, vext = kv_tiles[b]
                    UT = UT_sets[s2]
                    for h in range(H):
                        kt = kT[h // 2]
                        kq = kQT[h // 2]
                        pr = slice(64 * (h % 2), 64 * (h % 2) + 64)
                        up = upps.tile([D + 1, IQ], F32, name="uptile", tag="up")
                        for jg in range(JT // 4):  # groups of 4 key-tiles
                            dots = dotsps.tile(
                                [128, 4 * IQ], F32, name="dotstile", tag="dots"
                            )
                            for q in range(4):
                                jt = jg * 4 + q
                                qsl = slice(q * IQ, (q + 1) * IQ)
                                nc.tensor.matmul(
                                    dots[:, qsl],
                                    kt[pr, jt * 128 : (jt + 1) * 128],
                                    kq[pr, :],
                                    start=True,
                                    stop=False,
                                )
                                nc.tensor.matmul(
                                    dots[:, qsl],
                                    idm[:],
                                    posT_sb[jt][:],
                                    start=False,
                                    stop=True,
                                )
                            es = epool.tile(
                                [128, 4 * IQ], F32R, name="expS", tag="expS"
                            )
                            nc.scalar.activation(
                                es[:], dots[:], ACTF.Exp, scale=c_ap[:]
                            )
                            for q in range(4):
                                jt = jg * 4 + q
                                qsl = slice(q * IQ, (q + 1) * IQ)
                                nc.tensor.matmul(
                                    up[:],
                                    vext[jt][:, h, :],
                                    es[:, qsl],
                                    start=(jt == 0),
                                    stop=(jt == JT - 1),
                                )
                        # row 64 of up = Z; normalize U while copying out
                        nc.vector.reciprocal(rzrow[64:65, :], up[64:65, :])
                        nc.sync.dma_start(rz_bounce[:], rzrow[64:65, :])
                        nc.sync.dma_start(
                            rzb[:], rz_bounce[:].to_broadcast([64, IQ])
                        )
                        nc.vector.tensor_tensor(
                            UT[h][:], up[0:64, :], rzb[:], op=ALU.mult
                        )

                def emit_final(b):
                    s2 = b % 2
                    UT = UT_sets[s2]
                    for it in range(IQ // 128):
                        isl = slice(it * 128, (it + 1) * 128)
                        fps = mmps.tile([128, 512], F32, name="mmtile", tag="mm")
                        for h in range(H):
                            nc.tensor.matmul(
                                fps[:],
                                UT[h][:, isl],
                                wout[h][:],
                                start=(h == 0),
                                stop=False,
                            )
                        nc.tensor.matmul(
                            fps[:], ones1[:], bout[:], start=False, stop=True
                        )
                        ot = opool.tile([128, 512], F32, name="osb", tag="osb")
                        nc.vector.tensor_copy(ot[:], fps[:])
                        nc.sync.dma_start(y_d[b, isl, :], ot[:])

                emit_kv(0)
                emit_kv(1)
                emit_pos()
                emit_attn(0)
                emit_final(0)
                emit_kv(2)
                emit_attn(1)
                emit_final(1)
                emit_kv(3)
                emit_attn(2)
                emit_final(2)
                emit_attn(3)
                emit_final(3)

    nc.compile()
    return nc


_CACHE = {}


def _get_program():
    if "nc" not in _CACHE:
        _CACHE["nc"] = build_program()
    return _CACHE["nc"]


def _host_shard(x, pos, W_kv, W_out, b_out, w_pos, b_pos):
    """Build the 8 per-core input maps (pure layout work, no math)."""
    import ml_dtypes

    x = np.asarray(x, dtype=np.float32)
    pos = np.asarray(pos, dtype=np.float32)
    W_kv = np.asarray(W_kv, dtype=np.float32)
    W_out = np.asarray(W_out, dtype=np.float32)
    b_out = np.asarray(b_out, dtype=np.float32)
    w_pos = np.asarray(w_pos, dtype=np.float32)

    wkvT = np.ascontiguousarray(W_kv.T)  # (512, 1024)
    wout = np.ascontiguousarray(W_out.T)  # (512, 512)
    boutr = b_out.reshape(1, DIM)
    wposr = np.ascontiguousarray(
        np.broadcast_to(w_pos.astype(ml_dtypes.bfloat16), (128, POS_CHUNK, POS_DIM))
    )
    ones_arr = np.ones((128, 128), dtype=np.float32)
    id_arr = np.eye(128, dtype=ml_dtypes.bfloat16)

    in_maps = []
    for c in range(NC):
        g, s = c // 4, c % 4
        bs = slice(4 * g, 4 * g + BPC)
        isl = slice(s * IQ, (s + 1) * IQ)
        xT = np.ascontiguousarray(x[bs].transpose(0, 2, 1))  # (4, 512, 1024)
        xqT = np.ascontiguousarray(x[bs, isl].transpose(0, 2, 1))  # (4, 512, 256)
        posT = np.ascontiguousarray(
            pos[0, isl, :, :].transpose(1, 0, 2).astype(ml_dtypes.bfloat16)
        )  # (1024 j, 256 i, 50) bf16
        in_maps.append(
            {
                "xT": xT,
                "xqT": xqT,
                "wkvT": wkvT,
                "wout": wout,
                "bout": boutr,
                "wposr": wposr,
                "posT": posT,
                "ones": ones_arr,
                "idm": id_arr,
            }
        )
    return in_maps


def kernel(**inputs) -> np.ndarray:
    nc = _get_program()
    in_maps = _host_shard(**inputs)
    res = run_bass_kernel_spmd(nc, in_maps, list(range(NC)))
    out = np.empty((B, N, DIM), dtype=np.float32)
    for c in range(NC):
        g, s = c // 4, c % 4
        out[4 * g : 4 * g + BPC, s * IQ : (s + 1) * IQ, :] = res.results[c]["y"]
    return out


if __name__ == "__main__":
    import reference

    inputs = {k: np.asarray(v) for k, v in reference.setup_inputs().items()}
    expected = np.asarray(reference.reference(**inputs))
    actual = kernel(**inputs)
    err = np.abs(actual - expected).max()
    rel = err / np.abs(expected).max()
    print(f"absmax err: {err:.3e}  rel: {rel:.3e}")



# revision 34
# speedup vs baseline: 1.0126x; 1.0126x over previous
"""Trainium2 Bass kernel for nn_Attention_KV (dense transformer attention
with K=Q sharing and a linear positional bias), distributed over 8 cores.

Sharding: each core owns ALL 8 batches for one 128-row query octant.
The j (key) axis is rolled by -128*c per core on the host (pure layout),
so the SPMD-uniform program always finds its own query block in columns
0:128 of the on-device K^T — no per-core addressing, no shipped xq, and
pos ships exactly once across the 8 cores (i-octant slice, bf16).
Collectives are avoided entirely (~300us fixed latency each on this
fabric, measured in an earlier session).

Everything the PE touches is bf16 (f32 PSUM accumulation), which runs
1 cycle/row at any free size (f32r needs free>=256) and halves DMA +
SBUF. Attention keeps scores TRANSPOSED ([j, i]: keys on partitions):
  - dots^T lands in PSUM 4 heads at a time ([128 j, 4*128 i]); the
    pos bias is added by an identity-matmul whose rhs is pos_bias^T/c
    (pre-divided on device via wposr/c), so exp(scale=c) on the Scalar
    engine applies the dot scaling and the bias in one pass
  - attn@v: lhsT = v_ext (ones column appended -> row 64 of the result
    is the softmax denominator Z), rhs = exp(scores^T) bf16
  - normalization: Z row -> DVE reciprocal -> PE K=1 broadcast matmul
    ([1,64] ones x [1,512] rz -> [64,512] PSUM) -> DVE multiply, no
    DRAM bounce
  - output projection packs head pairs so K=128 (4 matmuls), bias via
    a K=1 ones x b_out matmul, per-batch [128 i, 512] f32 out
PSUM->SBUF copies are spread across Scalar (k^T), Pool (v, y) and DVE
(pos phase, normalize) so no single helper engine becomes critical.
b_pos (a scalar added to every score) is dropped: softmax is shift
invariant.
"""

import sys

sys.path.insert(0, "/opt/trn_rl_repo")

import numpy as np

import concourse.bacc as bacc
import concourse.bass as bass
import concourse.mybir as mybir
from concourse import tile
from concourse.bass_utils import run_bass_kernel_spmd

B, N, DIM, H, POS_DIM = 8, 1024, 512, 8, 50
D = DIM // H  # 64
NC = 8  # cores
IO = 128  # query rows per core (i-octant)
JT = N // 128  # 8 j-tiles
SCALE = float(DIM) ** -0.5

F32 = mybir.dt.float32
F32R = mybir.dt.float32r
BF16 = mybir.dt.bfloat16
AX = mybir.AxisListType
ALU = mybir.AluOpType
ACTF = mybir.ActivationFunctionType

POS_CHUNK = 64  # i-columns of pos processed per DVE reduce


def build_program():
    nc = bacc.Bacc("TRN2", target_bir_lowering=False, debug=False)

    # ---- DRAM parameters (per-core) ----
    xT_d = nc.declare_dram_parameter("xT", [B, DIM, N], BF16, isOutput=False)
    wkvT_d = nc.declare_dram_parameter("wkvT", [DIM, 2 * DIM], BF16, isOutput=False)
    wout_d = nc.declare_dram_parameter("wout", [H, D, DIM], BF16, isOutput=False)
    bout_d = nc.declare_dram_parameter("bout", [1, DIM], F32R, isOutput=False)
    wposr_d = nc.declare_dram_parameter(
        "wposr", [128, POS_CHUNK, POS_DIM], BF16, isOutput=False
    )
    posT_d = nc.declare_dram_parameter("posT", [N, IO, POS_DIM], BF16, isOutput=False)
    id_d = nc.declare_dram_parameter("idm", [128, 128], BF16, isOutput=False)
    ones_d = nc.declare_dram_parameter("ones", [65, 128], F32R, isOutput=False)
    ones16_d = nc.declare_dram_parameter("ones16", [128, H], BF16, isOutput=False)
    y_d = nc.declare_dram_parameter("y", [B, IO, DIM], F32, isOutput=True)

    with tile.TileContext(nc) as tc:
        with (
            tc.tile_pool(name="persist", bufs=1) as pp,
            tc.tile_pool(name="pos_in", bufs=2) as pos_pool,
            tc.tile_pool(name="exps", bufs=10) as epool,
            tc.tile_pool(name="outsb", bufs=2) as opool,
            tc.tile_pool(name="mm_ps", bufs=2, space="PSUM") as mmps,
            tc.tile_pool(name="dots_ps", bufs=2, space="PSUM") as dotsps,
            tc.tile_pool(name="up_ps", bufs=2, space="PSUM") as upps,
            tc.tile_pool(name="rz_ps", bufs=2, space="PSUM") as rzps,
        ):
            # ---- preload weights + small tensors ----
            wposr = pp.tile([128, POS_CHUNK, POS_DIM], BF16, tag="wposr")
            nc.sync.dma_start(wposr[:], wposr_d[:])
            idm = pp.tile([128, 128], BF16, tag="idm")
            nc.sync.dma_start(idm[:], id_d[:])
            wkvT = [
                pp.tile([128, 2 * DIM], BF16, name=f"wkvT{t}", tag=f"wkvT{t}")
                for t in range(4)
            ]
            for t in range(4):
                nc.sync.dma_start(wkvT[t][:], wkvT_d[t * 128 : (t + 1) * 128, :])
            wout = [
                pp.tile([D, DIM], BF16, name=f"wout{h}", tag=f"wout{h}")
                for h in range(H)
            ]
            for h in range(H):
                nc.sync.dma_start(wout[h][:], wout_d[h, :, :])
            bout = pp.tile([1, DIM], F32R, tag="bout")
            nc.sync.dma_start(bout[:], bout_d[:])

            # row 0: lhsT for the bias matmul; row 64: lhsT for the 1/Z
            # broadcast matmul — it sits at partition 64 to match the Z
            # row's PSUM partition (engines can't shift partitions, and
            # matmul requires lhsT/rhs at the same base partition).
            onesr = pp.tile([65, 128], F32R, tag="onesr")
            nc.sync.dma_start(onesr[:], ones_d[:])

            # c = scale * sum(w_pos) on every partition; wposr /= c so the
            # pos-bias accumulates pre-divided and exp(scale=c) restores it.
            c_ap = pp.tile([128, 1], F32, tag="c_ap")
            ic_ap = pp.tile([128, 1], F32, tag="ic_ap")
            nc.vector.tensor_reduce(c_ap[:], wposr[:, 0, :], axis=AX.X, op=ALU.add)
            nc.scalar.mul(c_ap[:], c_ap[:], SCALE)
            nc.vector.reciprocal(ic_ap[:], c_ap[:])
            with nc.allow_low_precision(reason="w_pos/c in bf16 is intended"):
                nc.vector.tensor_scalar_mul(wposr[:], wposr[:], ic_ap[:])

            # ---- per batch state ----
            xT_sets = {
                s2: [
                    pp.tile([128, N], BF16, name=f"xT{t}_{s2}", tag=f"xT{t}_{s2}")
                    for t in range(4)
                ]
                for s2 in (0, 1)
            }
            kT_sets = {
                s2: [
                    pp.tile([128, N], BF16, name=f"kT{t}_{s2}", tag=f"kT{t}_{s2}")
                    for t in range(4)
                ]
                for s2 in (0, 1)
            }
            vext_sets = {
                s2: [
                    pp.tile(
                        [128, H, D + 1],
                        BF16,
                        name=f"vext{t}_{s2}",
                        tag=f"vext{t}_{s2}",
                    )
                    for t in range(JT)
                ]
                for s2 in (0, 1)
            }
            usb_sets = {
                s2: [
                    pp.tile([64, IO], BF16, name=f"usb{h}_{s2}", tag=f"usb{h}_{s2}")
                    for h in range(H)
                ]
                for s2 in (0, 1)
            }
            # 1/Z staging; row 64 only (same partition as the PSUM Z row),
            # one 512-column block per head group
            rz_sets = {
                s2: pp.tile([65, 1024], F32R, name=f"rz_{s2}", tag=f"rz_{s2}")
                for s2 in (0, 1)
            }

            # vext ones columns survive across batches (the per-batch copy
            # only writes cols 0:D), so set them once at preload
            for s2 in (0, 1):
                for nt in range(JT):
                    nc.sync.dma_start(
                        vext_sets[s2][nt][:, :, D : D + 1], ones16_d[:, :]
                    )

            # posT_sb[jt] = pos_bias^T / c for key-tile jt (128 j x 128 i)
            posT_sb = [
                pp.tile([128, IO], BF16, name=f"posT{j}", tag=f"posT{j}")
                for j in range(JT)
            ]

            def emit_pos():
                for jt in range(JT):
                    for ic in range(IO // POS_CHUNK):
                        sl = slice(ic * POS_CHUNK, (ic + 1) * POS_CHUNK)
                        pt = pos_pool.tile(
                            [128, POS_CHUNK, POS_DIM], BF16, name="pchunk", tag="pchunk"
                        )
                        nc.sync.dma_start(
                            pt[:], posT_d[jt * 128 : (jt + 1) * 128, sl, :]
                        )
                        nc.vector.tensor_tensor(pt[:], pt[:], wposr[:], op=ALU.mult)
                        with nc.allow_low_precision(
                            reason="pos bias flows in bf16 by design"
                        ):
                            nc.vector.tensor_reduce(
                                posT_sb[jt][:, sl], pt[:], axis=AX.X, op=ALU.add
                            )

            def emit_kv(b):
                s2 = b % 2
                xT = xT_sets[s2]
                for t in range(4):
                    nc.sync.dma_start(xT[t][:], xT_d[b, t * 128 : (t + 1) * 128, :])
                kT = kT_sets[s2]
                for t in range(4):
                    for nchunk in range(2):
                        ps = mmps.tile([128, 512], F32, name="mmtile", tag="mm")
                        for dc in range(4):
                            nc.tensor.matmul(
                                ps[:],
                                wkvT[dc][:, t * 128 : (t + 1) * 128],
                                xT[dc][:, nchunk * 512 : (nchunk + 1) * 512],
                                start=(dc == 0),
                                stop=(dc == 3),
                            )
                        nc.scalar.copy(
                            kT[t][:, nchunk * 512 : (nchunk + 1) * 512], ps[:]
                        )
                vext = vext_sets[s2]
                for nt in range(JT):
                    ps = mmps.tile([128, 512], F32, name="mmtile", tag="mm")
                    for dc in range(4):
                        nc.tensor.matmul(
                            ps[:],
                            xT[dc][:, nt * 128 : (nt + 1) * 128],
                            wkvT[dc][:, DIM : 2 * DIM],
                            start=(dc == 0),
                            stop=(dc == 3),
                        )
                    nc.vector.tensor_copy(
                        vext[nt][:, :, 0:D],
                        ps[:].rearrange("p (h d) -> p h d", h=H),
                    )

            def emit_attn(b):
                s2 = b % 2
                kT = kT_sets[s2]
                vext = vext_sets[s2]
                usb = usb_sets[s2]
                rz = rz_sets[s2]
                for g in range(2):  # head groups of 4
                    up = upps.tile([D + 1, 512], F32, name="uptile", tag="up")
                    es_tiles = []
                    for jt in range(JT):
                        dots = dotsps.tile([128, 512], F32, name="dotstile", tag="dots")
                        for h4 in range(4):
                            h = 4 * g + h4
                            pr = slice(64 * (h % 2), 64 * (h % 2) + 64)
                            csl = slice(h4 * IO, (h4 + 1) * IO)
                            nc.tensor.matmul(
                                dots[:, csl],
                                kT[h // 2][pr, jt * 128 : (jt + 1) * 128],
                                kT[h // 2][pr, 0:IO],
                                start=True,
                                stop=False,
                            )
                            nc.tensor.matmul(
                                dots[:, csl],
                                idm[:],
                                posT_sb[jt][:],
                                start=False,
                                stop=True,
                            )
                        es = epool.tile([128, 512], BF16, name="expS", tag="expS")
                        nc.scalar.activation(es[:], dots[:], ACTF.Exp, scale=c_ap[:])
                        es_tiles.append(es)
                    # PSUM allows one open accumulation group per bank at a
                    # time, so each head's jt-chain must run start->stop
                    # consecutively (jt inner, head outer).
                    for h4 in range(4):
                        h = 4 * g + h4
                        csl = slice(h4 * IO, (h4 + 1) * IO)
                        for jt in range(JT):
                            nc.tensor.matmul(
                                up[:, csl],
                                vext[jt][:, h, :],
                                es_tiles[jt][:, csl],
                                start=(jt == 0),
                                stop=(jt == JT - 1),
                            )
                    # normalize: Z row -> recip -> PE broadcast -> DVE mult
                    gsl = slice(g * 512, (g + 1) * 512)
                    with nc.allow_low_precision(
                        reason="f32r is bit-identical to f32; matmul encoding only"
                    ):
                        nc.vector.reciprocal(rz[D : D + 1, gsl], up[D : D + 1, :])
                    rzb = rzps.tile([64, 512], F32, name="rzb", tag="rzb")
                    nc.tensor.matmul(
                        rzb[:],
                        onesr[D : D + 1, 0:64],
                        rz[D : D + 1, gsl],
                        start=True,
                        stop=True,
                    )
                    # DVE allows only one PSUM operand; stage 1/Z in SBUF
                    rzsb = epool.tile([64, 512], F32, name="rzsb", tag="rzsb")
                    nc.scalar.copy(rzsb[:], rzb[:])
                    for h4 in range(4):
                        h = 4 * g + h4
                        csl = slice(h4 * IO, (h4 + 1) * IO)
                        with nc.allow_low_precision(
                            reason="normalized attn output in bf16 by design"
                        ):
                            nc.vector.tensor_tensor(
                                usb[h][:],
                                up[0:D, csl],
                                rzsb[:, csl],
                                op=ALU.mult,
                            )

            def emit_final(b):
                s2 = b % 2
                usb = usb_sets[s2]
                fps = mmps.tile([128, 512], F32, name="mmtile", tag="mm")
                for h in range(H):
                    nc.tensor.matmul(
                        fps[:], usb[h][:], wout[h][:], start=(h == 0), stop=False
                    )
                nc.tensor.matmul(
                    fps[:], onesr[0:1, :], bout[:], start=False, stop=True
                )
                ot = opool.tile([128, 512], F32, name="osb", tag="osb")
                nc.scalar.copy(ot[:], fps[:])
                nc.sync.dma_start(y_d[b, :, :], ot[:])

            emit_kv(0)
            emit_pos()
            emit_kv(1)
            emit_attn(0)
            emit_final(0)
            for b in range(2, B):
                emit_kv(b)
                emit_attn(b - 1)
                emit_final(b - 1)
            emit_attn(B - 1)
            emit_final(B - 1)

    nc.compile()
    return nc


_CACHE = {}


def _get_program():
    if "nc" not in _CACHE:
        _CACHE["nc"] = build_program()
    return _CACHE["nc"]


def _host_shard(x, pos, W_kv, W_out, b_out, w_pos, b_pos):
    """Build the 8 per-core input maps (pure layout work, no math)."""
    import ml_dtypes

    bf16 = ml_dtypes.bfloat16
    x = np.asarray(x, dtype=np.float32)
    pos = np.asarray(pos, dtype=np.float32)
    W_kv = np.asarray(W_kv, dtype=np.float32)
    W_out = np.asarray(W_out, dtype=np.float32)
    b_out = np.asarray(b_out, dtype=np.float32)
    w_pos = np.asarray(w_pos, dtype=np.float32)

    xT = np.ascontiguousarray(x.transpose(0, 2, 1)).astype(bf16)  # (8, 512, 1024)
    wkvT = np.ascontiguousarray(W_kv.T).astype(bf16)  # (512, 1024)
    woutH = np.ascontiguousarray(W_out.T.reshape(H, D, DIM)).astype(bf16)
    boutr = b_out.reshape(1, DIM)
    wposr = np.ascontiguousarray(
        np.broadcast_to(w_pos.astype(bf16), (128, POS_CHUNK, POS_DIM))
    )
    id_arr = np.eye(128, dtype=bf16)
    ones_arr = np.ones((65, 128), dtype=np.float32)
    ones16_arr = np.ones((128, H), dtype=bf16)
    pos_bf = pos[0].astype(bf16)  # (1024 i, 1024 j, 50)

    in_maps = []
    for c in range(NC):
        s = c * IO
        isl = slice(s, s + IO)
        # roll x's sequence axis by -s so this core's queries are cols 0:128
        xTr = np.ascontiguousarray(
            np.concatenate([xT[:, :, s:], xT[:, :, :s]], axis=2)
        )
        pT = pos_bf[isl].transpose(1, 0, 2)  # (1024 j, 128 i, 50)
        posT = np.ascontiguousarray(np.concatenate([pT[s:], pT[:s]], axis=0))
        in_maps.append(
            {
                "xT": xTr,
                "wkvT": wkvT,
                "wout": woutH,
                "bout": boutr,
                "wposr": wposr,
                "posT": posT,
                "idm": id_arr,
                "ones": ones_arr,
                "ones16": ones16_arr,
            }
        )
    return in_maps


def kernel(**inputs) -> np.ndarray:
    nc = _get_program()
    in_maps = _host_shard(**inputs)
    res = run_bass_kernel_spmd(nc, in_maps, list(range(NC)))
    out = np.empty((B, N, DIM), dtype=np.float32)
    for c in range(NC):
        out[:, c * IO : (c + 1) * IO, :] = res.results[c]["y"]
    return out


if __name__ == "__main__":
    import reference

    inputs = {k: np.asarray(v) for k, v in reference.setup_inputs().items()}
    expected = np.asarray(reference.reference(**inputs))
    actual = kernel(**inputs)
    err = np.abs(actual - expected).max()
    rel = err / np.abs(expected).max()
    print(f"absmax err: {err:.3e}  rel: {rel:.3e}")
